# revision 11
# baseline (speedup 1.0000x reference)
"""BinaryAttention Trainium2 kernel: data-parallel over batch on 8 NeuronCores.

Per-core pipeline (16 batch items):
  qkvT = W^T-split-bf16 x3 matmul (q,k transposed d-major); v token-major 1-pass
  sign/abs via ScalarE from PSUM; per-(b,h) scale c = mean|q| mean|k| /8
  Attention in m-major (transposed) layout:
    S^T = sgnq @ sgnk per (head, m-tile) via PE (m on partitions);
    eps = exp(c*S^T) straight from PSUM (ScalarE, per-partition scale);
    eT = eps * exp(bias)  (DVE, two heads packed 394-wide on SBUF);
    Z = column sums via GPSIMD partition all-reduce + broadcast;
    pqT = RNE(255*eT/Z) -> fp16 ints; PV contracts m directly
    (no PE transposes, no evac-muls).
  v quantized with rs folded in (virs fp16); proj folds 1/255 into weights.
"""
import numpy as np
import ml_dtypes

import concourse.bacc as bacc
import concourse.mybir as mybir
import concourse.bass_isa as bass_isa
from concourse.tile import TileContext
from concourse.bass_utils import run_bass_kernel_spmd
from concourse.bass import AP
import concourse.bass as bass

N_CORES = 8
B = 128
BP = B // N_CORES          # 16 batch items per core
NT = 197                   # tokens
DIM = 768
NH = 12
HD = 64
NREL = 732
TOK = BP * NT              # 3152
F32 = mybir.dt.float32
BF16 = mybir.dt.bfloat16
F16 = mybir.dt.float16
bf = ml_dtypes.bfloat16
EXP2_23 = 8388608.0
C0 = 1.0 / (NT * HD) / (NT * HD) / 8.0

_CACHE = {}


def _build_nc():
    nc = bacc.Bacc("TRN2", target_bir_lowering=False, debug=False, num_devices=1)
    d = {}
    d["xh"] = nc.dram_tensor("xh", [DIM, TOK], BF16, kind="ExternalInput").ap()
    d["xl"] = nc.dram_tensor("xl", [DIM, TOK], BF16, kind="ExternalInput").ap()
    d["wh"] = nc.dram_tensor("wh", [DIM, 3 * DIM], BF16, kind="ExternalInput").ap()
    d["wl"] = nc.dram_tensor("wl", [DIM, 3 * DIM], BF16, kind="ExternalInput").ap()
    d["pw"] = nc.dram_tensor("pw", [DIM, DIM], BF16, kind="ExternalInput").ap()
    d["pb"] = nc.dram_tensor("pb", [DIM], F32, kind="ExternalInput").ap()
    # ebp[m, jp, hh*197+n] = exp(bias[2jp+hh, n, m]) (pair-packed, m-major)
    d["ebp"] = nc.dram_tensor("ebp", [NT, 6, 2 * NT], F32, kind="ExternalInput").ap()
    d["sel"] = nc.dram_tensor("sel", [128, 2], F32, kind="ExternalInput").ap()
    d["ones"] = nc.dram_tensor("ones", [128, 128], BF16, kind="ExternalInput").ap()
    d["out"] = nc.dram_tensor("out", [TOK, DIM], F32, kind="ExternalOutput").ap()
    cscr = nc.dram_tensor("cscr", [BP, 12], F32)

    ntl = [128, 69]   # m/n-tile sizes
    noff = [0, 128]

    with TileContext(nc) as tc:
        with (
            tc.tile_pool(name="singles", bufs=1) as singles,
            tc.tile_pool(name="xpool", bufs=2) as xpool,
            tc.tile_pool(name="bpool", bufs=2) as bpool,
            tc.tile_pool(name="hpool", bufs=2) as hpool,
            tc.tile_pool(name="psA", bufs=2, space="PSUM") as psA,
            tc.tile_pool(name="psS", bufs=1, space="PSUM") as psS,
            tc.tile_pool(name="psP", bufs=1, space="PSUM") as psP,
            tc.tile_pool(name="psZ", bufs=1, space="PSUM") as psZ,
        ):
            # ---- resident weights/constants ----
            whs = singles.tile([128, 6, 3 * DIM], BF16, tag="whs")
            wls = singles.tile([128, 6, 3 * DIM], BF16, tag="wls")
            nc.sync.dma_start(out=whs[:], in_=d["wh"].rearrange("(k p) n -> p k n", p=128))
            nc.sync.dma_start(out=wls[:], in_=d["wl"].rearrange("(k p) n -> p k n", p=128))
            pws = singles.tile([128, 6, DIM], BF16, tag="pws")
            nc.sync.dma_start(out=pws[:], in_=d["pw"].rearrange("(k p) n -> p k n", p=128))
            ebp0 = singles.tile([128, 6, 2 * NT], F32, tag="ebp0")
            ebp1 = singles.tile([128, 6, 2 * NT], F32, tag="ebp1")
            nc.sync.dma_start(out=ebp0[:], in_=d["ebp"][0:128])
            nc.sync.dma_start(out=ebp1[:69], in_=d["ebp"][128:NT])
            ones = singles.tile([128, 128], BF16, tag="ones")
            nc.sync.dma_start(out=ones[:], in_=d["ones"])
            pbs = singles.tile([128, DIM], F32, tag="pbs")
            nc.gpsimd.dma_start(out=pbs[:], in_=AP(tensor=d["pb"].tensor, offset=0, ap=[[0, 128], [1, DIM]]))
            sels = singles.tile([128, 2], F32, tag="sels")
            nc.sync.dma_start(out=sels[:], in_=d["sel"])

            ebp = [ebp0, ebp1]

            for bb in range(BP // 2):   # pairs of batch items
                c2 = 2 * NT
                xh_t = xpool.tile([128, 6, c2], BF16, tag="xh")
                xl_t = xpool.tile([128, 6, c2], BF16, tag="xl")
                nc.sync.dma_start(out=xh_t[:], in_=d["xh"].rearrange("(k p) t -> p k t", p=128)[:, :, bb * c2:(bb + 1) * c2])
                nc.sync.dma_start(out=xl_t[:], in_=d["xl"].rearrange("(k p) t -> p k t", p=128)[:, :, bb * c2:(bb + 1) * c2])

                sgn = [bpool.tile([128, NH, NT], BF16, tag=f"sgn{i}", name=f"sgn{i}") for i in range(2)]
                absc = [bpool.tile([128, 12], F32, tag=f"absc{i}", name=f"absc{i}") for i in range(2)]
                dump = [bpool.tile([128, NT], BF16, tag=f"dump{i}", name=f"dump{i}") for i in range(2)]

                # ---- stage A: q,k transposed (12 j-tiles of 128 rows), 3-pass ----
                for j in range(12):
                    pa = psA.tile([128, c2], F32, tag="A")
                    for k in range(6):
                        wj = slice(j * 128, (j + 1) * 128)
                        first = (k == 0)
                        nc.tensor.matmul(pa[:], whs[:, k, wj], xh_t[:, k, :], start=first, stop=False)
                        nc.tensor.matmul(pa[:], whs[:, k, wj], xl_t[:, k, :], start=False, stop=False)
                        nc.tensor.matmul(pa[:], wls[:, k, wj], xh_t[:, k, :], start=False, stop=(k == 5))
                    for i in range(2):
                        sl = slice(i * NT, (i + 1) * NT)
                        nc.scalar.activation(out=sgn[i][:, j, :], in_=pa[:, sl], func=mybir.ActivationFunctionType.Sign)
                        nc.scalar.activation(out=dump[i][:], in_=pa[:, sl], func=mybir.ActivationFunctionType.Abs,
                                             accum_out=absc[i][:, j:j + 1])

                virs = [[None, None], [None, None]]
                cbc = [None, None]
                attnT = [None, None]
                for i in range(2):
                    b = bb * 2 + i
                    # ---- v natural (1-pass) + quantization; rs folded in ----
                    for t in range(2):
                        virs[i][t] = bpool.tile([128, DIM], F16, tag=f"virs{i}{t}", name=f"virs{i}{t}")
                        tn = ntl[t]
                        xoff = i * NT + noff[t]
                        pv = psA.tile([128, 384], F32, tag="A", name="pv")
                        vq32 = bpool.tile([128, 384], F32, tag="vq32")
                        vmax = bpool.tile([128, 12], F32, tag="vmax")
                        rsb = bpool.tile([128, 12], F32, tag="rsb")
                        ss = bpool.tile([128, 12], F32, tag="ss")
                        for ch in range(2):
                            vj = slice(1536 + ch * 384, 1536 + (ch + 1) * 384)
                            for k in range(6):
                                nc.tensor.matmul(pv[:tn], xh_t[:, k, xoff:xoff + tn], whs[:, k, vj],
                                                 start=(k == 0), stop=(k == 5))
                            hs = slice(ch * 6, (ch + 1) * 6)
                            # clip to [-2,2]
                            nc.vector.tensor_scalar(out=vq32[:tn], in0=pv[:tn], scalar1=2.0, scalar2=-2.0,
                                                    op0=mybir.AluOpType.min, op1=mybir.AluOpType.max)
                            # row max |.| per head
                            nc.vector.tensor_reduce(out=vmax[:tn, hs], in_=vq32[:tn].rearrange("p (h d) -> p h d", h=6),
                                                    axis=mybir.AxisListType.X, op=mybir.AluOpType.max,
                                                    apply_absolute_value=True)
                            # rs = (max+1e-8)/127 ; ss = 127/(max+1e-8)
                            nc.vector.tensor_scalar(out=rsb[:tn, hs], in0=vmax[:tn, hs], scalar1=1e-8, scalar2=1.0 / 127.0,
                                                    op0=mybir.AluOpType.add, op1=mybir.AluOpType.mult)
                            nc.vector.reciprocal(out=ss[:tn, hs], in_=rsb[:tn, hs])
                            # v*ss, RNE round via +-2^23 (in-place), then * rs -> fp16
                            sbase = ss[:tn, hs]
                            sbc = AP(tensor=sbase.tensor, offset=sbase.offset,
                                     ap=[[int(s), int(c)] for s, c in sbase.ap] + [[0, HD]])
                            rbase = rsb[:tn, hs]
                            rbc = AP(tensor=rbase.tensor, offset=rbase.offset,
                                     ap=[[int(s), int(c)] for s, c in rbase.ap] + [[0, HD]])
                            v3 = vq32[:tn].rearrange("p (h d) -> p h d", h=6)
                            nc.vector.tensor_tensor(out=v3, in0=v3, in1=sbc, op=mybir.AluOpType.mult)
                            nc.vector.tensor_scalar(out=vq32[:tn], in0=vq32[:tn], scalar1=EXP2_23, scalar2=EXP2_23,
                                                    op0=mybir.AluOpType.add, op1=mybir.AluOpType.subtract)
                            nc.vector.tensor_tensor(out=virs[i][t][:tn, ch * 384:(ch + 1) * 384].rearrange("p (h d) -> p h d", h=6),
                                                    in0=v3, in1=rbc, op=mybir.AluOpType.mult)
                    # ---- c stats -> cbc[p, h] = c(b, h) broadcast over partitions ----
                    cst = psP.tile([2, 12], F32, tag="P", name="cst")
                    nc.tensor.matmul(cst[:], sels[:], absc[i][:], start=True, stop=True)
                    css = bpool.tile([2, 12], F32, tag="css")
                    nc.vector.tensor_copy(css[:], cst[:])
                    csb = bpool.tile([2, 6], F32, tag="csb")
                    nc.vector.tensor_tensor(out=csb[:], in0=css[:2, 0:6], in1=css[:2, 6:12], op=mybir.AluOpType.mult)
                    nc.vector.tensor_scalar_mul(csb[:], csb[:], C0)
                    nc.sync.dma_start(out=cscr.ap()[b].rearrange("(r j) -> r j", r=2), in_=csb[:])
                    cbc[i] = bpool.tile([128, 12], F32, tag=f"cbc{i}", name=f"cbc{i}")
                    nc.gpsimd.dma_start(out=cbc[i][:], in_=AP(tensor=cscr, offset=b * 12, ap=[[0, 128], [1, 12]]))
                    attnT[i] = bpool.tile([128, 6, NT], BF16, tag=f"attnT{i}", name=f"attnT{i}")

                # ---- attention per (head-pair, batch item) ----
                for jp in range(6):
                    for i in range(2):
                        eT = [hpool.tile([128, c2], F32, tag=f"eT{mt}", name=f"eT{mt}") for mt in range(2)]
                        for mt in range(2):
                            mc = ntl[mt]
                            for hh in range(2):
                                h = 2 * jp + hh
                                base = 64 * hh
                                ps1 = psS.tile([128, NT], F32, tag=f"S{hh}{mt}", name=f"ps{hh}{mt}")
                                # cidx mapping: head h -> cbc column (h%2)*6 + h//2
                                cidx = (h % 2) * 6 + h // 2
                                nc.tensor.matmul(ps1[:mc],
                                                 sgn[i][base:base + 64, 6 + jp, noff[mt]:noff[mt] + mc],
                                                 sgn[i][base:base + 64, jp, :],
                                                 start=True, stop=True)
                                # eps = exp(c * S^T) straight from PSUM
                                nc.scalar.activation(out=eT[mt][:mc, hh * NT:(hh + 1) * NT], in_=ps1[:mc],
                                                     func=mybir.ActivationFunctionType.Exp,
                                                     scale=cbc[i][:mc, cidx:cidx + 1])
                            # eT *= exp(bias) (paired, 394-wide SBUF)
                            nc.vector.tensor_tensor(out=eT[mt][:mc], in0=eT[mt][:mc],
                                                    in1=ebp[mt][:mc, jp, :], op=mybir.AluOpType.mult)
                        # eT16 = bf16(eT/255) on gpsimd; Zb = ones^T @ eT16
                        # -> Z/255 replicated on all partitions; rzb = 255/Z
                        eT16 = [hpool.tile([128, c2], BF16, tag=f"eT16{mt}", name=f"eT16{mt}") for mt in range(2)]
                        pz = psZ.tile([128, c2], F32, tag="Z", name="pz")
                        for mt in range(2):
                            mc = ntl[mt]
                            nc.gpsimd.tensor_scalar(out=eT16[mt][:mc], in0=eT[mt][:mc],
                                                    scalar1=1.0 / 255.0, scalar2=None,
                                                    op0=mybir.AluOpType.mult)
                            nc.tensor.matmul(pz[:], ones[:mc, :], eT16[mt][:mc],
                                             start=(mt == 0), stop=(mt == 1))
                        rzb = hpool.tile([128, c2], F32, tag="rzb")
                        nc.vector.reciprocal_approx_fast(out=rzb[:], in_=pz[:])
                        # pqT = RNE(eT * rzb) -> fp16 (exact ints)
                        pqT = [hpool.tile([128, c2], F16, tag=f"pqT{mt}", name=f"pqT{mt}") for mt in range(2)]
                        for mt in range(2):
                            mc = ntl[mt]
                            y2 = hpool.tile([128, c2], F32, tag=f"y2{mt}", name=f"y2{mt}")
                            nc.gpsimd.tensor_tensor(out=y2[:mc], in0=eT[mt][:mc], in1=rzb[:mc],
                                                    op=mybir.AluOpType.mult)
                            nc.vector.tensor_scalar(out=pqT[mt][:mc], in0=y2[:mc], scalar1=EXP2_23, scalar2=EXP2_23,
                                                    op0=mybir.AluOpType.add, op1=mybir.AluOpType.subtract)
                        # PV: both heads into one [128, NT] bank (partition-split)
                        ppv = psP.tile([128, NT], F32, tag="P", name="ppv")
                        for hh in range(2):
                            h = 2 * jp + hh
                            for mt in range(2):
                                mc = ntl[mt]
                                nc.tensor.matmul(ppv[64 * hh:64 * hh + 64, :],
                                                 virs[i][mt][:mc, h * 64:(h + 1) * 64],
                                                 pqT[mt][:mc, hh * NT:(hh + 1) * NT],
                                                 start=(mt == 0), stop=(mt == 1))
                        nc.scalar.activation(out=attnT[i][:, jp, :], in_=ppv[:],
                                             func=mybir.ActivationFunctionType.Copy)
                # ---- proj ----
                for i in range(2):
                    b = bb * 2 + i
                    tb = b * NT
                    osb = [bpool.tile([128, DIM], F32, tag=f"osb{t}", name=f"osb{t}") for t in range(2)]
                    for t in range(2):
                        tn = ntl[t]
                        for ch in range(2):
                            pp = psA.tile([128, 384], F32, tag="A", name="pp")
                            for jt in range(6):
                                nc.tensor.matmul(pp[:tn], attnT[i][:, jt, noff[t]:noff[t] + tn],
                                                 pws[:, jt, ch * 384:(ch + 1) * 384], start=(jt == 0), stop=(jt == 5))
                            nc.vector.scalar_tensor_tensor(out=osb[t][:tn, ch * 384:(ch + 1) * 384], in0=pp[:tn],
                                                           scalar=1.0, in1=pbs[:tn, ch * 384:(ch + 1) * 384],
                                                           op0=mybir.AluOpType.mult, op1=mybir.AluOpType.add)
                        nc.sync.dma_start(out=d["out"][tb + noff[t]:tb + noff[t] + tn, :], in_=osb[t][:tn])
    nc.compile()
    return nc


def kernel(x, qkv_w, proj_w, proj_b, rel_bias_table, rel_index):
    x = np.asarray(x, dtype=np.float32)
    qkv_w = np.asarray(qkv_w, dtype=np.float32)
    proj_w = np.asarray(proj_w, dtype=np.float32)
    proj_b = np.asarray(proj_b, dtype=np.float32)
    rel_bias_table = np.asarray(rel_bias_table, dtype=np.float32)
    rel_index = np.asarray(rel_index)

    if "nc" not in _CACHE:
        _CACHE["nc"] = _build_nc()
    nc = _CACHE["nc"]

    W2 = np.ascontiguousarray(qkv_w.T)                      # (768, 2304)
    wh = W2.astype(bf)
    wl = (W2 - wh.astype(np.float32)).astype(bf)
    pw = np.ascontiguousarray(proj_w.T / 255.0).astype(bf)  # fold 1/255
    biasg = rel_bias_table[rel_index].astype(np.float32)    # (197,197,12) [n,m,h]
    # ebp[m, jp, hh*197+n] = exp(bias[h=2jp+hh, n, m])
    ebp = np.ascontiguousarray(
        np.exp(biasg).transpose(1, 2, 0).reshape(NT, 6, 2 * NT))
    sel = np.zeros((128, 2), np.float32)
    sel[:64, 0] = 1.0
    sel[64:, 1] = 1.0
    onesc = np.ones((128, 128), dtype=bf)

    in_maps = []
    for c in range(N_CORES):
        xc = x[c * BP:(c + 1) * BP].reshape(TOK, DIM)
        xT = np.ascontiguousarray(xc.T)                     # (768, 3152)
        xh = xT.astype(bf)
        xl = (xT - xh.astype(np.float32)).astype(bf)
        in_maps.append({
            "xh": xh, "xl": xl, "wh": wh, "wl": wl, "pw": pw,
            "pb": proj_b.astype(np.float32), "ebp": ebp,
            "sel": sel, "ones": onesc,
        })

    global _LAST_IN_MAPS
    _LAST_IN_MAPS = in_maps
    res = run_bass_kernel_spmd(nc, in_maps, list(range(N_CORES)))
    out = np.concatenate(
        [res.results[c]["out"].reshape(BP, NT, DIM) for c in range(N_CORES)], axis=0)
    return out.astype(np.float32)


# revision 13
# speedup vs baseline: 2.2258x; 2.2258x over previous
"""BinaryAttention Trainium2 kernel: data-parallel over batch on 8 NeuronCores.

Per-core pipeline (16 batch items):
  qkvT = W^T-split-bf16 x3 matmul (q,k transposed d-major); v token-major 1-pass
  sign/abs via ScalarE from PSUM; per-(b,h) scale c = mean|q| mean|k| /8
  Attention in m-major (transposed) layout:
    S^T = sgnq @ sgnk per (head, m-tile) via PE (m on partitions);
    eps = exp(c*S^T) straight from PSUM (ScalarE, per-partition scale);
    eT = eps * exp(bias)  (DVE, two heads packed 394-wide on SBUF);
    Z = column sums via GPSIMD partition all-reduce + broadcast;
    pqT = RNE(255*eT/Z) -> fp16 ints; PV contracts m directly
    (no PE transposes, no evac-muls).
  v quantized with rs folded in (virs fp16); proj folds 1/255 into weights.
"""
import numpy as np
import ml_dtypes

import concourse.bacc as bacc
import concourse.mybir as mybir
import concourse.bass_isa as bass_isa
from concourse.tile import TileContext
from concourse.bass_utils import run_bass_kernel_spmd
from concourse.bass import AP
import concourse.bass as bass

N_CORES = 8
B = 128
BP = B // N_CORES          # 16 batch items per core
NT = 197                   # tokens
DIM = 768
NH = 12
HD = 64
NREL = 732
TOK = BP * NT              # 3152
F32 = mybir.dt.float32
BF16 = mybir.dt.bfloat16
F16 = mybir.dt.float16
bf = ml_dtypes.bfloat16
EXP2_23 = 8388608.0
C0 = 1.0 / (NT * HD) / (NT * HD) / 8.0

_CACHE = {}


def _build_nc():
    nc = bacc.Bacc("TRN2", target_bir_lowering=False, debug=False, num_devices=1)
    d = {}
    d["xh"] = nc.dram_tensor("xh", [DIM, TOK], BF16, kind="ExternalInput").ap()
    d["xl"] = nc.dram_tensor("xl", [DIM, TOK], BF16, kind="ExternalInput").ap()
    d["wh"] = nc.dram_tensor("wh", [DIM, 3 * DIM], BF16, kind="ExternalInput").ap()
    d["wl"] = nc.dram_tensor("wl", [DIM, 3 * DIM], BF16, kind="ExternalInput").ap()
    d["pw"] = nc.dram_tensor("pw", [DIM, DIM], BF16, kind="ExternalInput").ap()
    d["pb"] = nc.dram_tensor("pb", [DIM], F32, kind="ExternalInput").ap()
    # ebp[m, jp, hh*197+n] = exp(bias[2jp+hh, n, m]) (pair-packed, m-major)
    d["ebp"] = nc.dram_tensor("ebp", [NT, 6, 2 * NT], F32, kind="ExternalInput").ap()
    d["sel"] = nc.dram_tensor("sel", [128, 2], F32, kind="ExternalInput").ap()
    d["ones"] = nc.dram_tensor("ones", [128, 128], F16, kind="ExternalInput").ap()
    d["out"] = nc.dram_tensor("out", [TOK, DIM], F32, kind="ExternalOutput").ap()
    cscr = nc.dram_tensor("cscr", [BP, 12], F32)

    ntl = [128, 69]   # m/n-tile sizes
    noff = [0, 128]

    with TileContext(nc) as tc:
        with (
            tc.tile_pool(name="singles", bufs=1) as singles,
            tc.tile_pool(name="xpool", bufs=2) as xpool,
            tc.tile_pool(name="bpool", bufs=2) as bpool,
            tc.tile_pool(name="hpool", bufs=2) as hpool,
            tc.tile_pool(name="psA", bufs=2, space="PSUM") as psA,
            tc.tile_pool(name="psS", bufs=1, space="PSUM") as psS,
            tc.tile_pool(name="psP", bufs=1, space="PSUM") as psP,
            tc.tile_pool(name="psZ", bufs=1, space="PSUM") as psZ,
        ):
            # ---- resident weights/constants ----
            whs = singles.tile([128, 6, 3 * DIM], BF16, tag="whs")
            wls = singles.tile([128, 6, 3 * DIM], BF16, tag="wls")
            nc.sync.dma_start(out=whs[:], in_=d["wh"].rearrange("(k p) n -> p k n", p=128))
            nc.sync.dma_start(out=wls[:], in_=d["wl"].rearrange("(k p) n -> p k n", p=128))
            pws = singles.tile([128, 6, DIM], BF16, tag="pws")
            nc.sync.dma_start(out=pws[:], in_=d["pw"].rearrange("(k p) n -> p k n", p=128))
            ebp0 = singles.tile([128, 6, 2 * NT], F32, tag="ebp0")
            ebp1 = singles.tile([128, 6, 2 * NT], F32, tag="ebp1")
            nc.sync.dma_start(out=ebp0[:], in_=d["ebp"][0:128])
            nc.sync.dma_start(out=ebp1[:69], in_=d["ebp"][128:NT])
            ones = singles.tile([128, 128], F16, tag="ones")
            nc.sync.dma_start(out=ones[:], in_=d["ones"])
            pbs = singles.tile([128, DIM], F32, tag="pbs")
            nc.gpsimd.dma_start(out=pbs[:], in_=AP(tensor=d["pb"].tensor, offset=0, ap=[[0, 128], [1, DIM]]))
            sels = singles.tile([128, 2], F32, tag="sels")
            nc.sync.dma_start(out=sels[:], in_=d["sel"])

            ebp = [ebp0, ebp1]

            for bb in range(BP // 2):   # pairs of batch items
                c2 = 2 * NT
                xh_t = xpool.tile([128, 6, c2], BF16, tag="xh")
                xl_t = xpool.tile([128, 6, c2], BF16, tag="xl")
                nc.sync.dma_start(out=xh_t[:], in_=d["xh"].rearrange("(k p) t -> p k t", p=128)[:, :, bb * c2:(bb + 1) * c2])
                nc.sync.dma_start(out=xl_t[:], in_=d["xl"].rearrange("(k p) t -> p k t", p=128)[:, :, bb * c2:(bb + 1) * c2])

                sgn = [bpool.tile([128, NH, NT], BF16, tag=f"sgn{i}", name=f"sgn{i}") for i in range(2)]
                absc = [bpool.tile([128, 12], F32, tag=f"absc{i}", name=f"absc{i}") for i in range(2)]
                dump = [bpool.tile([128, NT], BF16, tag=f"dump{i}", name=f"dump{i}") for i in range(2)]

                # ---- stage A: q,k transposed (12 j-tiles of 128 rows), 3-pass ----
                for j in range(12):
                    pa = psA.tile([128, c2], F32, tag="A")
                    for k in range(6):
                        wj = slice(j * 128, (j + 1) * 128)
                        first = (k == 0)
                        nc.tensor.matmul(pa[:], whs[:, k, wj], xh_t[:, k, :], start=first, stop=False)
                        nc.tensor.matmul(pa[:], whs[:, k, wj], xl_t[:, k, :], start=False, stop=False)
                        nc.tensor.matmul(pa[:], wls[:, k, wj], xh_t[:, k, :], start=False, stop=(k == 5))
                    for i in range(2):
                        sl = slice(i * NT, (i + 1) * NT)
                        nc.scalar.activation(out=sgn[i][:, j, :], in_=pa[:, sl], func=mybir.ActivationFunctionType.Sign)
                        nc.scalar.activation(out=dump[i][:], in_=pa[:, sl], func=mybir.ActivationFunctionType.Abs,
                                             accum_out=absc[i][:, j:j + 1])

                virs = [[None, None], [None, None]]
                cbc = [None, None]
                attnT = [None, None]
                for i in range(2):
                    b = bb * 2 + i
                    # ---- v natural (1-pass) + quantization; rs folded in ----
                    for t in range(2):
                        virs[i][t] = bpool.tile([128, DIM], F16, tag=f"virs{i}{t}", name=f"virs{i}{t}")
                        tn = ntl[t]
                        xoff = i * NT + noff[t]
                        pv = psA.tile([128, 384], F32, tag="A", name="pv")
                        vq32 = bpool.tile([128, 384], F32, tag="vq32")
                        vmax = bpool.tile([128, 12], F32, tag="vmax")
                        rsb = bpool.tile([128, 12], F32, tag="rsb")
                        ss = bpool.tile([128, 12], F32, tag="ss")
                        for ch in range(2):
                            vj = slice(1536 + ch * 384, 1536 + (ch + 1) * 384)
                            for k in range(6):
                                nc.tensor.matmul(pv[:tn], xh_t[:, k, xoff:xoff + tn], whs[:, k, vj],
                                                 start=(k == 0), stop=(k == 5))
                            hs = slice(ch * 6, (ch + 1) * 6)
                            # clip to [-2,2]
                            nc.vector.tensor_scalar(out=vq32[:tn], in0=pv[:tn], scalar1=2.0, scalar2=-2.0,
                                                    op0=mybir.AluOpType.min, op1=mybir.AluOpType.max)
                            # row max |.| per head
                            nc.vector.tensor_reduce(out=vmax[:tn, hs], in_=vq32[:tn].rearrange("p (h d) -> p h d", h=6),
                                                    axis=mybir.AxisListType.X, op=mybir.AluOpType.max,
                                                    apply_absolute_value=True)
                            # rs = (max+1e-8)/127 ; ss = 127/(max+1e-8)
                            nc.vector.tensor_scalar(out=rsb[:tn, hs], in0=vmax[:tn, hs], scalar1=1e-8, scalar2=1.0 / 127.0,
                                                    op0=mybir.AluOpType.add, op1=mybir.AluOpType.mult)
                            nc.vector.reciprocal(out=ss[:tn, hs], in_=rsb[:tn, hs])
                            # v*ss, RNE round via +-2^23 (in-place), then * rs -> fp16
                            sbase = ss[:tn, hs]
                            sbc = AP(tensor=sbase.tensor, offset=sbase.offset,
                                     ap=[[int(s), int(c)] for s, c in sbase.ap] + [[0, HD]])
                            rbase = rsb[:tn, hs]
                            rbc = AP(tensor=rbase.tensor, offset=rbase.offset,
                                     ap=[[int(s), int(c)] for s, c in rbase.ap] + [[0, HD]])
                            v3 = vq32[:tn].rearrange("p (h d) -> p h d", h=6)
                            nc.vector.tensor_tensor(out=v3, in0=v3, in1=sbc, op=mybir.AluOpType.mult)
                            nc.vector.tensor_scalar(out=vq32[:tn], in0=vq32[:tn], scalar1=EXP2_23, scalar2=EXP2_23,
                                                    op0=mybir.AluOpType.add, op1=mybir.AluOpType.subtract)
                            nc.vector.tensor_tensor(out=virs[i][t][:tn, ch * 384:(ch + 1) * 384].rearrange("p (h d) -> p h d", h=6),
                                                    in0=v3, in1=rbc, op=mybir.AluOpType.mult)
                    # ---- c stats -> cbc[p, h] = c(b, h) broadcast over partitions ----
                    cst = psP.tile([2, 12], F32, tag="P", name="cst")
                    nc.tensor.matmul(cst[:], sels[:], absc[i][:], start=True, stop=True)
                    css = bpool.tile([2, 12], F32, tag="css")
                    nc.vector.tensor_copy(css[:], cst[:])
                    csb = bpool.tile([2, 6], F32, tag="csb")
                    nc.vector.tensor_tensor(out=csb[:], in0=css[:2, 0:6], in1=css[:2, 6:12], op=mybir.AluOpType.mult)
                    nc.vector.tensor_scalar_mul(csb[:], csb[:], C0)
                    nc.sync.dma_start(out=cscr.ap()[b].rearrange("(r j) -> r j", r=2), in_=csb[:])
                    cbc[i] = bpool.tile([128, 12], F32, tag=f"cbc{i}", name=f"cbc{i}")
                    nc.gpsimd.dma_start(out=cbc[i][:], in_=AP(tensor=cscr, offset=b * 12, ap=[[0, 128], [1, 12]]))
                    attnT[i] = bpool.tile([128, 6, NT], BF16, tag=f"attnT{i}", name=f"attnT{i}")

                # ---- attention per (head-pair, batch item), software-pipelined ----
                # stage1(u): S^T matmuls -> exp(c*S^T) f32 -> *exp(bias) -> f16 eF
                # stage2(u-LAG): Z = onesT(1/255) @ eF -> rzb=recipfast ->
                #                pqT=RNE(eF*rzb) -> PV -> evac
                units = [(jp, i) for jp in range(6) for i in range(2)]
                LAG = 2
                stash = {}
                for u in range(len(units) + LAG):
                    if u < len(units):
                        jp, i = units[u]
                        eps = [hpool.tile([128, c2], F32, tag=f"eps{mt}", name=f"eps{mt}", bufs=2) for mt in range(2)]
                        eF = [hpool.tile([128, c2], F16, tag=f"eF{mt}", name=f"eF{mt}", bufs=LAG + 1) for mt in range(2)]
                        for mt in range(2):
                            mc = ntl[mt]
                            for hh in range(2):
                                h = 2 * jp + hh
                                base = 64 * hh
                                ps1 = psS.tile([128, NT], F32, tag=f"S{hh}{mt}", name=f"ps{hh}{mt}")
                                # cidx mapping: head h -> cbc column (h%2)*6 + h//2
                                cidx = (h % 2) * 6 + h // 2
                                nc.tensor.matmul(ps1[:mc],
                                                 sgn[i][base:base + 64, 6 + jp, noff[mt]:noff[mt] + mc],
                                                 sgn[i][base:base + 64, jp, :],
                                                 start=True, stop=True)
                                # eps = exp(c * S^T) straight from PSUM
                                nc.scalar.activation(out=eps[mt][:mc, hh * NT:(hh + 1) * NT], in_=ps1[:mc],
                                                     func=mybir.ActivationFunctionType.Exp,
                                                     scale=cbc[i][:mc, cidx:cidx + 1])
                            # eF = f16(eps * exp(bias)) (paired, 394-wide SBUF)
                            nc.vector.tensor_tensor(out=eF[mt][:mc], in0=eps[mt][:mc],
                                                    in1=ebp[mt][:mc, jp, :], op=mybir.AluOpType.mult)
                        stash[u] = eF
                    if u >= LAG:
                        jp, i = units[u - LAG]
                        eF = stash.pop(u - LAG)
                        # Zb = ones(1/255)^T @ eF -> Z/255 replicated on all partitions
                        pz = psZ.tile([128, c2], F32, tag="Z", name="pz")
                        for mt in range(2):
                            mc = ntl[mt]
                            nc.tensor.matmul(pz[:], ones[:mc, :], eF[mt][:mc],
                                             start=(mt == 0), stop=(mt == 1))
                        rzb = hpool.tile([128, c2], F32, tag="rzb")
                        nc.vector.reciprocal_approx_fast(out=rzb[:], in_=pz[:])
                        # pqT = RNE(eF * rzb) -> fp16 (exact ints)
                        pqT = [hpool.tile([128, c2], F16, tag=f"pqT{mt}", name=f"pqT{mt}") for mt in range(2)]
                        for mt in range(2):
                            mc = ntl[mt]
                            y2 = hpool.tile([128, c2], F32, tag=f"y2{mt}", name=f"y2{mt}")
                            nc.vector.tensor_tensor(out=y2[:mc], in0=eF[mt][:mc], in1=rzb[:mc],
                                                    op=mybir.AluOpType.mult)
                            nc.vector.tensor_scalar(out=pqT[mt][:mc], in0=y2[:mc], scalar1=EXP2_23, scalar2=EXP2_23,
                                                    op0=mybir.AluOpType.add, op1=mybir.AluOpType.subtract)
                        # PV: both heads into one [128, NT] bank (partition-split)
                        ppv = psP.tile([128, NT], F32, tag="P", name="ppv")
                        for hh in range(2):
                            h = 2 * jp + hh
                            for mt in range(2):
                                mc = ntl[mt]
                                nc.tensor.matmul(ppv[64 * hh:64 * hh + 64, :],
                                                 virs[i][mt][:mc, h * 64:(h + 1) * 64],
                                                 pqT[mt][:mc, hh * NT:(hh + 1) * NT],
                                                 start=(mt == 0), stop=(mt == 1))
                        nc.scalar.activation(out=attnT[i][:, jp, :], in_=ppv[:],
                                             func=mybir.ActivationFunctionType.Copy)
                # ---- proj ----
                for i in range(2):
                    b = bb * 2 + i
                    tb = b * NT
                    osb = [bpool.tile([128, DIM], F32, tag=f"osb{t}", name=f"osb{t}") for t in range(2)]
                    for t in range(2):
                        tn = ntl[t]
                        for ch in range(2):
                            pp = psA.tile([128, 384], F32, tag="A", name="pp")
                            for jt in range(6):
                                nc.tensor.matmul(pp[:tn], attnT[i][:, jt, noff[t]:noff[t] + tn],
                                                 pws[:, jt, ch * 384:(ch + 1) * 384], start=(jt == 0), stop=(jt == 5))
                            nc.vector.scalar_tensor_tensor(out=osb[t][:tn, ch * 384:(ch + 1) * 384], in0=pp[:tn],
                                                           scalar=1.0, in1=pbs[:tn, ch * 384:(ch + 1) * 384],
                                                           op0=mybir.AluOpType.mult, op1=mybir.AluOpType.add)
                        nc.sync.dma_start(out=d["out"][tb + noff[t]:tb + noff[t] + tn, :], in_=osb[t][:tn])
    nc.compile()
    return nc


def kernel(x, qkv_w, proj_w, proj_b, rel_bias_table, rel_index):
    x = np.asarray(x, dtype=np.float32)
    qkv_w = np.asarray(qkv_w, dtype=np.float32)
    proj_w = np.asarray(proj_w, dtype=np.float32)
    proj_b = np.asarray(proj_b, dtype=np.float32)
    rel_bias_table = np.asarray(rel_bias_table, dtype=np.float32)
    rel_index = np.asarray(rel_index)

    if "nc" not in _CACHE:
        _CACHE["nc"] = _build_nc()
    nc = _CACHE["nc"]

    W2 = np.ascontiguousarray(qkv_w.T)                      # (768, 2304)
    wh = W2.astype(bf)
    wl = (W2 - wh.astype(np.float32)).astype(bf)
    pw = np.ascontiguousarray(proj_w.T / 255.0).astype(bf)  # fold 1/255
    biasg = rel_bias_table[rel_index].astype(np.float32)    # (197,197,12) [n,m,h]
    # ebp[m, jp, hh*197+n] = exp(bias[h=2jp+hh, n, m])
    ebp = np.ascontiguousarray(
        np.exp(biasg).transpose(1, 2, 0).reshape(NT, 6, 2 * NT))
    sel = np.zeros((128, 2), np.float32)
    sel[:64, 0] = 1.0
    sel[64:, 1] = 1.0
    onesc = np.full((128, 128), 1.0 / 255.0, dtype=np.float16)

    in_maps = []
    for c in range(N_CORES):
        xc = x[c * BP:(c + 1) * BP].reshape(TOK, DIM)
        xT = np.ascontiguousarray(xc.T)                     # (768, 3152)
        xh = xT.astype(bf)
        xl = (xT - xh.astype(np.float32)).astype(bf)
        in_maps.append({
            "xh": xh, "xl": xl, "wh": wh, "wl": wl, "pw": pw,
            "pb": proj_b.astype(np.float32), "ebp": ebp,
            "sel": sel, "ones": onesc,
        })

    global _LAST_IN_MAPS
    _LAST_IN_MAPS = in_maps
    res = run_bass_kernel_spmd(nc, in_maps, list(range(N_CORES)))
    out = np.concatenate(
        [res.results[c]["out"].reshape(BP, NT, DIM) for c in range(N_CORES)], axis=0)
    return out.astype(np.float32)


# revision 14
# speedup vs baseline: 2.2848x; 1.0265x over previous
"""BinaryAttention Trainium2 kernel: data-parallel over batch on 8 NeuronCores.

Per-core pipeline (16 batch items):
  qkvT = W^T-split-bf16 x3 matmul (q,k transposed d-major); v token-major 1-pass
  sign/abs via ScalarE from PSUM; per-(b,h) scale c = mean|q| mean|k| /8
  Attention in m-major (transposed) layout:
    S^T = sgnq @ sgnk per (head, m-tile) via PE (m on partitions);
    eps = exp(c*S^T) straight from PSUM (ScalarE, per-partition scale);
    eT = eps * exp(bias)  (DVE, two heads packed 394-wide on SBUF);
    Z = column sums via GPSIMD partition all-reduce + broadcast;
    pqT = RNE(255*eT/Z) -> fp16 ints; PV contracts m directly
    (no PE transposes, no evac-muls).
  v quantized with rs folded in (virs fp16); proj folds 1/255 into weights.
"""
import numpy as np
import ml_dtypes

import concourse.bacc as bacc
import concourse.mybir as mybir
import concourse.bass_isa as bass_isa
from concourse.tile import TileContext
from concourse.bass_utils import run_bass_kernel_spmd
from concourse.bass import AP
import concourse.bass as bass

N_CORES = 8
B = 128
BP = B // N_CORES          # 16 batch items per core
NT = 197                   # tokens
DIM = 768
NH = 12
HD = 64
NREL = 732
TOK = BP * NT              # 3152
F32 = mybir.dt.float32
BF16 = mybir.dt.bfloat16
F16 = mybir.dt.float16
bf = ml_dtypes.bfloat16
EXP2_23 = 8388608.0
C0 = 1.0 / (NT * HD) / (NT * HD) / 8.0

_CACHE = {}


def _build_nc():
    nc = bacc.Bacc("TRN2", target_bir_lowering=False, debug=False, num_devices=1)
    d = {}
    d["xh"] = nc.dram_tensor("xh", [DIM, TOK], BF16, kind="ExternalInput").ap()
    d["xl"] = nc.dram_tensor("xl", [DIM, TOK], BF16, kind="ExternalInput").ap()
    d["wh"] = nc.dram_tensor("wh", [DIM, 3 * DIM], BF16, kind="ExternalInput").ap()
    d["wl"] = nc.dram_tensor("wl", [DIM, 3 * DIM], BF16, kind="ExternalInput").ap()
    d["pw"] = nc.dram_tensor("pw", [DIM, DIM], BF16, kind="ExternalInput").ap()
    d["pb"] = nc.dram_tensor("pb", [DIM], F32, kind="ExternalInput").ap()
    # ebp[m, jp, hh*197+n] = exp(bias[2jp+hh, n, m]) (pair-packed, m-major)
    d["ebp"] = nc.dram_tensor("ebp", [NT, 6, 2 * NT], F32, kind="ExternalInput").ap()
    d["sel"] = nc.dram_tensor("sel", [128, 2], F32, kind="ExternalInput").ap()
    d["ones"] = nc.dram_tensor("ones", [128, 128], F16, kind="ExternalInput").ap()
    d["out"] = nc.dram_tensor("out", [TOK, DIM], F32, kind="ExternalOutput").ap()
    cscr = nc.dram_tensor("cscr", [BP, 12], F32)

    ntl = [128, 69]   # m/n-tile sizes
    noff = [0, 128]

    with TileContext(nc) as tc:
        with (
            tc.tile_pool(name="singles", bufs=1) as singles,
            tc.tile_pool(name="xpool", bufs=2) as xpool,
            tc.tile_pool(name="bpool", bufs=2) as bpool,
            tc.tile_pool(name="hpool", bufs=2) as hpool,
            tc.tile_pool(name="psS", bufs=1, space="PSUM") as psS,
            tc.tile_pool(name="psP", bufs=2, space="PSUM") as psP,
            tc.tile_pool(name="psZ", bufs=2, space="PSUM") as psZ,
        ):
            # ---- resident weights/constants ----
            whs = singles.tile([128, 6, 3 * DIM], BF16, tag="whs")
            wls = singles.tile([128, 6, 3 * DIM], BF16, tag="wls")
            nc.sync.dma_start(out=whs[:], in_=d["wh"].rearrange("(k p) n -> p k n", p=128))
            nc.sync.dma_start(out=wls[:], in_=d["wl"].rearrange("(k p) n -> p k n", p=128))
            pws = singles.tile([128, 6, DIM], BF16, tag="pws")
            nc.sync.dma_start(out=pws[:], in_=d["pw"].rearrange("(k p) n -> p k n", p=128))
            ebp0 = singles.tile([128, 6, 2 * NT], F32, tag="ebp0")
            ebp1 = singles.tile([128, 6, 2 * NT], F32, tag="ebp1")
            nc.sync.dma_start(out=ebp0[:], in_=d["ebp"][0:128])
            nc.sync.dma_start(out=ebp1[:69], in_=d["ebp"][128:NT])
            ones = singles.tile([128, 128], F16, tag="ones")
            nc.sync.dma_start(out=ones[:], in_=d["ones"])
            pbs = singles.tile([128, DIM], F32, tag="pbs")
            nc.gpsimd.dma_start(out=pbs[:], in_=AP(tensor=d["pb"].tensor, offset=0, ap=[[0, 128], [1, DIM]]))
            sels = singles.tile([128, 2], F32, tag="sels")
            nc.sync.dma_start(out=sels[:], in_=d["sel"])

            ebp = [ebp0, ebp1]

            for bb in range(BP // 2):   # pairs of batch items
                c2 = 2 * NT
                xh_t = xpool.tile([128, 6, c2], BF16, tag="xh")
                xl_t = xpool.tile([128, 6, c2], BF16, tag="xl")
                nc.sync.dma_start(out=xh_t[:], in_=d["xh"].rearrange("(k p) t -> p k t", p=128)[:, :, bb * c2:(bb + 1) * c2])
                nc.sync.dma_start(out=xl_t[:], in_=d["xl"].rearrange("(k p) t -> p k t", p=128)[:, :, bb * c2:(bb + 1) * c2])

                sgn = [bpool.tile([128, NH, NT], BF16, tag=f"sgn{i}", name=f"sgn{i}") for i in range(2)]
                absc = [bpool.tile([128, 12], F32, tag=f"absc{i}", name=f"absc{i}") for i in range(2)]
                dump = [bpool.tile([128, NT], BF16, tag=f"dump{i}", name=f"dump{i}") for i in range(2)]

                # ---- stage A: q,k transposed (12 j-tiles of 128 rows), 3-pass ----
                for j in range(12):
                    pa = psS.tile([128, c2], F32, tag=f"S{j % 2}0", name="pa")
                    for k in range(6):
                        wj = slice(j * 128, (j + 1) * 128)
                        first = (k == 0)
                        nc.tensor.matmul(pa[:], whs[:, k, wj], xh_t[:, k, :], start=first, stop=False)
                        nc.tensor.matmul(pa[:], whs[:, k, wj], xl_t[:, k, :], start=False, stop=False)
                        nc.tensor.matmul(pa[:], wls[:, k, wj], xh_t[:, k, :], start=False, stop=(k == 5))
                    for i in range(2):
                        sl = slice(i * NT, (i + 1) * NT)
                        nc.scalar.activation(out=sgn[i][:, j, :], in_=pa[:, sl], func=mybir.ActivationFunctionType.Sign)
                        nc.scalar.activation(out=dump[i][:], in_=pa[:, sl], func=mybir.ActivationFunctionType.Abs,
                                             accum_out=absc[i][:, j:j + 1])

                virs = [[None, None], [None, None]]
                cbc = [None, None]
                attnT = [None, None]
                for i in range(2):
                    b = bb * 2 + i
                    # ---- v natural (1-pass) + quantization; rs folded in ----
                    for t in range(2):
                        virs[i][t] = bpool.tile([128, DIM], F16, tag=f"virs{i}{t}", name=f"virs{i}{t}")
                        tn = ntl[t]
                        xoff = i * NT + noff[t]
                        pv = psS.tile([128, 384], F32, tag=f"S{t % 2}1", name="pv")
                        vq32 = bpool.tile([128, 384], F32, tag="vq32")
                        vmax = bpool.tile([128, 12], F32, tag="vmax")
                        rsb = bpool.tile([128, 12], F32, tag="rsb")
                        ss = bpool.tile([128, 12], F32, tag="ss")
                        for ch in range(2):
                            vj = slice(1536 + ch * 384, 1536 + (ch + 1) * 384)
                            for k in range(6):
                                nc.tensor.matmul(pv[:tn], xh_t[:, k, xoff:xoff + tn], whs[:, k, vj],
                                                 start=(k == 0), stop=(k == 5))
                            hs = slice(ch * 6, (ch + 1) * 6)
                            # clip to [-2,2]
                            nc.vector.tensor_scalar(out=vq32[:tn], in0=pv[:tn], scalar1=2.0, scalar2=-2.0,
                                                    op0=mybir.AluOpType.min, op1=mybir.AluOpType.max)
                            # row max |.| per head
                            nc.vector.tensor_reduce(out=vmax[:tn, hs], in_=vq32[:tn].rearrange("p (h d) -> p h d", h=6),
                                                    axis=mybir.AxisListType.X, op=mybir.AluOpType.max,
                                                    apply_absolute_value=True)
                            # rs = (max+1e-8)/127 ; ss = 127/(max+1e-8)
                            nc.vector.tensor_scalar(out=rsb[:tn, hs], in0=vmax[:tn, hs], scalar1=1e-8, scalar2=1.0 / 127.0,
                                                    op0=mybir.AluOpType.add, op1=mybir.AluOpType.mult)
                            nc.vector.reciprocal(out=ss[:tn, hs], in_=rsb[:tn, hs])
                            # v*ss, RNE round via +-2^23 (in-place), then * rs -> fp16
                            sbase = ss[:tn, hs]
                            sbc = AP(tensor=sbase.tensor, offset=sbase.offset,
                                     ap=[[int(s), int(c)] for s, c in sbase.ap] + [[0, HD]])
                            rbase = rsb[:tn, hs]
                            rbc = AP(tensor=rbase.tensor, offset=rbase.offset,
                                     ap=[[int(s), int(c)] for s, c in rbase.ap] + [[0, HD]])
                            v3 = vq32[:tn].rearrange("p (h d) -> p h d", h=6)
                            nc.vector.tensor_tensor(out=v3, in0=v3, in1=sbc, op=mybir.AluOpType.mult)
                            nc.vector.tensor_scalar(out=vq32[:tn], in0=vq32[:tn], scalar1=EXP2_23, scalar2=EXP2_23,
                                                    op0=mybir.AluOpType.add, op1=mybir.AluOpType.subtract)
                            nc.vector.tensor_tensor(out=virs[i][t][:tn, ch * 384:(ch + 1) * 384].rearrange("p (h d) -> p h d", h=6),
                                                    in0=v3, in1=rbc, op=mybir.AluOpType.mult)
                    # ---- c stats -> cbc[p, h] = c(b, h) broadcast over partitions ----
                    cst = psP.tile([2, 12], F32, tag="P", name="cst")
                    nc.tensor.matmul(cst[:], sels[:], absc[i][:], start=True, stop=True)
                    css = bpool.tile([2, 12], F32, tag="css")
                    nc.vector.tensor_copy(css[:], cst[:])
                    csb = bpool.tile([2, 6], F32, tag="csb")
                    nc.vector.tensor_tensor(out=csb[:], in0=css[:2, 0:6], in1=css[:2, 6:12], op=mybir.AluOpType.mult)
                    nc.vector.tensor_scalar_mul(csb[:], csb[:], C0)
                    nc.sync.dma_start(out=cscr.ap()[b].rearrange("(r j) -> r j", r=2), in_=csb[:])
                    cbc[i] = bpool.tile([128, 12], F32, tag=f"cbc{i}", name=f"cbc{i}")
                    nc.gpsimd.dma_start(out=cbc[i][:], in_=AP(tensor=cscr, offset=b * 12, ap=[[0, 128], [1, 12]]))
                    attnT[i] = bpool.tile([128, 6, NT], BF16, tag=f"attnT{i}", name=f"attnT{i}")

                # ---- attention per (head-pair, batch item), software-pipelined ----
                # stage1(u): S^T matmuls -> exp(c*S^T) f32 -> *exp(bias) -> f16 eF
                # stage2(u-LAG): Z = onesT(1/255) @ eF -> rzb=recipfast ->
                #                pqT=RNE(eF*rzb) -> PV -> evac
                units = [(jp, i) for jp in range(6) for i in range(2)]
                LAG = 3
                stash = {}
                for u in range(len(units) + LAG):
                    if u < len(units):
                        jp, i = units[u]
                        eps = [hpool.tile([128, c2], F32, tag=f"eps{mt}", name=f"eps{mt}", bufs=2) for mt in range(2)]
                        eF = [hpool.tile([128, c2], F16, tag=f"eF{mt}", name=f"eF{mt}", bufs=LAG + 1) for mt in range(2)]
                        for mt in range(2):
                            mc = ntl[mt]
                            for hh in range(2):
                                h = 2 * jp + hh
                                base = 64 * hh
                                ps1 = psS.tile([128, NT], F32, tag=f"S{hh}{mt}", name=f"ps{hh}{mt}")
                                # cidx mapping: head h -> cbc column (h%2)*6 + h//2
                                cidx = (h % 2) * 6 + h // 2
                                nc.tensor.matmul(ps1[:mc],
                                                 sgn[i][base:base + 64, 6 + jp, noff[mt]:noff[mt] + mc],
                                                 sgn[i][base:base + 64, jp, :],
                                                 start=True, stop=True)
                                # eps = exp(c * S^T) straight from PSUM
                                nc.scalar.activation(out=eps[mt][:mc, hh * NT:(hh + 1) * NT], in_=ps1[:mc],
                                                     func=mybir.ActivationFunctionType.Exp,
                                                     scale=cbc[i][:mc, cidx:cidx + 1])
                            # eF = f16(eps * exp(bias)) (paired, 394-wide SBUF)
                            nc.vector.tensor_tensor(out=eF[mt][:mc], in0=eps[mt][:mc],
                                                    in1=ebp[mt][:mc, jp, :], op=mybir.AluOpType.mult)
                        stash[u] = eF
                    if u >= LAG:
                        jp, i = units[u - LAG]
                        eF = stash.pop(u - LAG)
                        # Zb = ones(1/255)^T @ eF -> Z/255 replicated on all partitions
                        pz = psZ.tile([128, c2], F32, tag="Z", name="pz")
                        for mt in range(2):
                            mc = ntl[mt]
                            nc.tensor.matmul(pz[:], ones[:mc, :], eF[mt][:mc],
                                             start=(mt == 0), stop=(mt == 1))
                        rzb = hpool.tile([128, c2], F32, tag="rzb")
                        nc.vector.reciprocal_approx_fast(out=rzb[:], in_=pz[:])
                        # pqT = RNE(eF * rzb) -> fp16 (exact ints)
                        pqT = [hpool.tile([128, c2], F16, tag=f"pqT{mt}", name=f"pqT{mt}") for mt in range(2)]
                        for mt in range(2):
                            mc = ntl[mt]
                            y2 = hpool.tile([128, c2], F32, tag=f"y2{mt}", name=f"y2{mt}")
                            nc.vector.tensor_tensor(out=y2[:mc], in0=eF[mt][:mc], in1=rzb[:mc],
                                                    op=mybir.AluOpType.mult)
                            nc.vector.tensor_scalar(out=pqT[mt][:mc], in0=y2[:mc], scalar1=EXP2_23, scalar2=EXP2_23,
                                                    op0=mybir.AluOpType.add, op1=mybir.AluOpType.subtract)
                        # PV: both heads into one [128, NT] bank (partition-split)
                        ppv = psP.tile([128, NT], F32, tag="P", name="ppv")
                        for hh in range(2):
                            h = 2 * jp + hh
                            for mt in range(2):
                                mc = ntl[mt]
                                nc.tensor.matmul(ppv[64 * hh:64 * hh + 64, :],
                                                 virs[i][mt][:mc, h * 64:(h + 1) * 64],
                                                 pqT[mt][:mc, hh * NT:(hh + 1) * NT],
                                                 start=(mt == 0), stop=(mt == 1))
                        nc.scalar.activation(out=attnT[i][:, jp, :], in_=ppv[:],
                                             func=mybir.ActivationFunctionType.Copy)
                # ---- proj ----
                for i in range(2):
                    b = bb * 2 + i
                    tb = b * NT
                    osb = [bpool.tile([128, DIM], F32, tag=f"osb{t}", name=f"osb{t}") for t in range(2)]
                    for t in range(2):
                        tn = ntl[t]
                        for ch in range(2):
                            pp = psS.tile([128, 384], F32, tag=f"S{ch % 2}1", name="pp")
                            for jt in range(6):
                                nc.tensor.matmul(pp[:tn], attnT[i][:, jt, noff[t]:noff[t] + tn],
                                                 pws[:, jt, ch * 384:(ch + 1) * 384], start=(jt == 0), stop=(jt == 5))
                            nc.vector.scalar_tensor_tensor(out=osb[t][:tn, ch * 384:(ch + 1) * 384], in0=pp[:tn],
                                                           scalar=1.0, in1=pbs[:tn, ch * 384:(ch + 1) * 384],
                                                           op0=mybir.AluOpType.mult, op1=mybir.AluOpType.add)
                        nc.sync.dma_start(out=d["out"][tb + noff[t]:tb + noff[t] + tn, :], in_=osb[t][:tn])
    nc.compile()
    return nc


def kernel(x, qkv_w, proj_w, proj_b, rel_bias_table, rel_index):
    x = np.asarray(x, dtype=np.float32)
    qkv_w = np.asarray(qkv_w, dtype=np.float32)
    proj_w = np.asarray(proj_w, dtype=np.float32)
    proj_b = np.asarray(proj_b, dtype=np.float32)
    rel_bias_table = np.asarray(rel_bias_table, dtype=np.float32)
    rel_index = np.asarray(rel_index)

    if "nc" not in _CACHE:
        _CACHE["nc"] = _build_nc()
    nc = _CACHE["nc"]

    W2 = np.ascontiguousarray(qkv_w.T)                      # (768, 2304)
    wh = W2.astype(bf)
    wl = (W2 - wh.astype(np.float32)).astype(bf)
    pw = np.ascontiguousarray(proj_w.T / 255.0).astype(bf)  # fold 1/255
    biasg = rel_bias_table[rel_index].astype(np.float32)    # (197,197,12) [n,m,h]
    # ebp[m, jp, hh*197+n] = exp(bias[h=2jp+hh, n, m])
    ebp = np.ascontiguousarray(
        np.exp(biasg).transpose(1, 2, 0).reshape(NT, 6, 2 * NT))
    sel = np.zeros((128, 2), np.float32)
    sel[:64, 0] = 1.0
    sel[64:, 1] = 1.0
    onesc = np.full((128, 128), 1.0 / 255.0, dtype=np.float16)

    in_maps = []
    for c in range(N_CORES):
        xc = x[c * BP:(c + 1) * BP].reshape(TOK, DIM)
        xT = np.ascontiguousarray(xc.T)                     # (768, 3152)
        xh = xT.astype(bf)
        xl = (xT - xh.astype(np.float32)).astype(bf)
        in_maps.append({
            "xh": xh, "xl": xl, "wh": wh, "wl": wl, "pw": pw,
            "pb": proj_b.astype(np.float32), "ebp": ebp,
            "sel": sel, "ones": onesc,
        })

    global _LAST_IN_MAPS
    _LAST_IN_MAPS = in_maps
    res = run_bass_kernel_spmd(nc, in_maps, list(range(N_CORES)))
    out = np.concatenate(
        [res.results[c]["out"].reshape(BP, NT, DIM) for c in range(N_CORES)], axis=0)
    return out.astype(np.float32)


# revision 15
# speedup vs baseline: 2.3575x; 1.0318x over previous
"""BinaryAttention Trainium2 kernel: data-parallel over batch on 8 NeuronCores.

Per-core pipeline (16 batch items):
  qkvT = W^T-split-bf16 x3 matmul (q,k transposed d-major); v token-major 1-pass
  sign/abs via ScalarE from PSUM; per-(b,h) scale c = mean|q| mean|k| /8
  Attention in m-major (transposed) layout:
    S^T = sgnq @ sgnk per (head, m-tile) via PE (m on partitions);
    eps = exp(c*S^T) straight from PSUM (ScalarE, per-partition scale);
    eT = eps * exp(bias)  (DVE, two heads packed 394-wide on SBUF);
    Z = column sums via GPSIMD partition all-reduce + broadcast;
    pqT = RNE(255*eT/Z) -> fp16 ints; PV contracts m directly
    (no PE transposes, no evac-muls).
  v quantized with rs folded in (virs fp16); proj folds 1/255 into weights.
"""
import numpy as np
import ml_dtypes

import concourse.bacc as bacc
import concourse.mybir as mybir
import concourse.bass_isa as bass_isa
from concourse.tile import TileContext
from concourse.bass_utils import run_bass_kernel_spmd
from concourse.bass import AP
import concourse.bass as bass

N_CORES = 8
B = 128
BP = B // N_CORES          # 16 batch items per core
NT = 197                   # tokens
DIM = 768
NH = 12
HD = 64
NREL = 732
TOK = BP * NT              # 3152
F32 = mybir.dt.float32
BF16 = mybir.dt.bfloat16
F16 = mybir.dt.float16
bf = ml_dtypes.bfloat16
EXP2_23 = 8388608.0
C0 = 1.0 / (NT * HD) / (NT * HD) / 8.0

_CACHE = {}


def _build_nc():
    nc = bacc.Bacc("TRN2", target_bir_lowering=False, debug=False, num_devices=1)
    d = {}
    d["xh"] = nc.dram_tensor("xh", [DIM, TOK], BF16, kind="ExternalInput").ap()
    d["xl"] = nc.dram_tensor("xl", [DIM, TOK], BF16, kind="ExternalInput").ap()
    d["wh"] = nc.dram_tensor("wh", [DIM, 3 * DIM], BF16, kind="ExternalInput").ap()
    d["wl"] = nc.dram_tensor("wl", [DIM, 3 * DIM], BF16, kind="ExternalInput").ap()
    d["pw"] = nc.dram_tensor("pw", [DIM, DIM], BF16, kind="ExternalInput").ap()
    d["pb"] = nc.dram_tensor("pb", [DIM], F32, kind="ExternalInput").ap()
    # ebp[m, jp, hh*197+n] = exp(bias[2jp+hh, n, m]) (pair-packed, m-major)
    d["ebp"] = nc.dram_tensor("ebp", [NT, 6, 2 * NT], F32, kind="ExternalInput").ap()
    d["sel"] = nc.dram_tensor("sel", [128, 2], F32, kind="ExternalInput").ap()
    d["ones"] = nc.dram_tensor("ones", [128, 128], F16, kind="ExternalInput").ap()
    d["out"] = nc.dram_tensor("out", [TOK, DIM], F32, kind="ExternalOutput").ap()
    cscr = nc.dram_tensor("cscr", [BP, 12], F32)

    ntl = [128, 69]   # m/n-tile sizes
    noff = [0, 128]

    with TileContext(nc) as tc:
        with (
            tc.tile_pool(name="singles", bufs=1) as singles,
            tc.tile_pool(name="xpool", bufs=2) as xpool,
            tc.tile_pool(name="bpool", bufs=2) as bpool,
            tc.tile_pool(name="hpool", bufs=2) as hpool,
            tc.tile_pool(name="psS", bufs=1, space="PSUM") as psS,
            tc.tile_pool(name="psP", bufs=2, space="PSUM") as psP,
            tc.tile_pool(name="psZ", bufs=2, space="PSUM") as psZ,
        ):
            # ---- resident weights/constants ----
            whs = singles.tile([128, 6, 3 * DIM], BF16, tag="whs")
            wls = singles.tile([128, 6, 3 * DIM], BF16, tag="wls")
            nc.sync.dma_start(out=whs[:], in_=d["wh"].rearrange("(k p) n -> p k n", p=128))
            nc.sync.dma_start(out=wls[:], in_=d["wl"].rearrange("(k p) n -> p k n", p=128))
            pws = singles.tile([128, 6, DIM], BF16, tag="pws")
            nc.sync.dma_start(out=pws[:], in_=d["pw"].rearrange("(k p) n -> p k n", p=128))
            ebp0 = singles.tile([128, 6, 2 * NT], F32, tag="ebp0")
            ebp1 = singles.tile([128, 6, 2 * NT], F32, tag="ebp1")
            nc.sync.dma_start(out=ebp0[:], in_=d["ebp"][0:128])
            nc.sync.dma_start(out=ebp1[:69], in_=d["ebp"][128:NT])
            ones = singles.tile([128, 128], F16, tag="ones")
            nc.sync.dma_start(out=ones[:], in_=d["ones"])
            pbs = singles.tile([128, DIM], F32, tag="pbs")
            nc.gpsimd.dma_start(out=pbs[:], in_=AP(tensor=d["pb"].tensor, offset=0, ap=[[0, 128], [1, DIM]]))
            sels = singles.tile([128, 2], F32, tag="sels")
            nc.sync.dma_start(out=sels[:], in_=d["sel"])

            ebp = [ebp0, ebp1]

            for bb in range(BP // 2):   # pairs of batch items
                c2 = 2 * NT
                xh_t = xpool.tile([128, 6, c2], BF16, tag="xh")
                xl_t = xpool.tile([128, 6, c2], BF16, tag="xl")
                nc.sync.dma_start(out=xh_t[:], in_=d["xh"].rearrange("(k p) t -> p k t", p=128)[:, :, bb * c2:(bb + 1) * c2])
                nc.sync.dma_start(out=xl_t[:], in_=d["xl"].rearrange("(k p) t -> p k t", p=128)[:, :, bb * c2:(bb + 1) * c2])

                sgn = [bpool.tile([128, NH, NT], BF16, tag=f"sgn{i}", name=f"sgn{i}") for i in range(2)]
                absc = [bpool.tile([128, 12], F32, tag=f"absc{i}", name=f"absc{i}") for i in range(2)]
                dump = [bpool.tile([128, NT], BF16, tag=f"dump{i}", name=f"dump{i}") for i in range(2)]

                # ---- stage A: q,k transposed (12 j-tiles of 128 rows), 3-pass ----
                for j in range(12):
                    pa = psS.tile([128, c2], F32, tag=f"S{j % 2}0", name="pa")
                    for k in range(6):
                        wj = slice(j * 128, (j + 1) * 128)
                        first = (k == 0)
                        nc.tensor.matmul(pa[:], whs[:, k, wj], xh_t[:, k, :], start=first, stop=False)
                        nc.tensor.matmul(pa[:], whs[:, k, wj], xl_t[:, k, :], start=False, stop=False)
                        nc.tensor.matmul(pa[:], wls[:, k, wj], xh_t[:, k, :], start=False, stop=(k == 5))
                    for i in range(2):
                        sl = slice(i * NT, (i + 1) * NT)
                        nc.scalar.activation(out=sgn[i][:, j, :], in_=pa[:, sl], func=mybir.ActivationFunctionType.Sign)
                        nc.scalar.activation(out=dump[i][:], in_=pa[:, sl], func=mybir.ActivationFunctionType.Abs,
                                             accum_out=absc[i][:, j:j + 1])

                virs = [[None, None], [None, None]]
                cbc = [None, None]
                attnT = [None, None]
                for i in range(2):
                    b = bb * 2 + i
                    # ---- v natural (1-pass) + quantization; rs folded in ----
                    for t in range(2):
                        virs[i][t] = bpool.tile([128, DIM], F16, tag=f"virs{i}{t}", name=f"virs{i}{t}")
                        tn = ntl[t]
                        xoff = i * NT + noff[t]
                        pv = psS.tile([128, 384], F32, tag=f"S{t % 2}1", name="pv")
                        vq32 = bpool.tile([128, 384], F32, tag="vq32")
                        vmax = bpool.tile([128, 12], F32, tag="vmax")
                        rsb = bpool.tile([128, 12], F32, tag="rsb")
                        ss = bpool.tile([128, 12], F32, tag="ss")
                        for ch in range(2):
                            vj = slice(1536 + ch * 384, 1536 + (ch + 1) * 384)
                            for k in range(6):
                                nc.tensor.matmul(pv[:tn], xh_t[:, k, xoff:xoff + tn], whs[:, k, vj],
                                                 start=(k == 0), stop=(k == 5))
                            hs = slice(ch * 6, (ch + 1) * 6)
                            # clip to [-2,2]
                            nc.vector.tensor_scalar(out=vq32[:tn], in0=pv[:tn], scalar1=2.0, scalar2=-2.0,
                                                    op0=mybir.AluOpType.min, op1=mybir.AluOpType.max)
                            # row max |.| per head
                            nc.vector.tensor_reduce(out=vmax[:tn, hs], in_=vq32[:tn].rearrange("p (h d) -> p h d", h=6),
                                                    axis=mybir.AxisListType.X, op=mybir.AluOpType.max,
                                                    apply_absolute_value=True)
                            # rs = (max+1e-8)/127 ; ss = 127/(max+1e-8)
                            nc.vector.tensor_scalar(out=rsb[:tn, hs], in0=vmax[:tn, hs], scalar1=1e-8, scalar2=1.0 / 127.0,
                                                    op0=mybir.AluOpType.add, op1=mybir.AluOpType.mult)
                            nc.vector.reciprocal(out=ss[:tn, hs], in_=rsb[:tn, hs])
                            # v*ss, RNE round via +-2^23 (in-place), then * rs -> fp16
                            sbase = ss[:tn, hs]
                            sbc = AP(tensor=sbase.tensor, offset=sbase.offset,
                                     ap=[[int(s), int(c)] for s, c in sbase.ap] + [[0, HD]])
                            rbase = rsb[:tn, hs]
                            rbc = AP(tensor=rbase.tensor, offset=rbase.offset,
                                     ap=[[int(s), int(c)] for s, c in rbase.ap] + [[0, HD]])
                            v3 = vq32[:tn].rearrange("p (h d) -> p h d", h=6)
                            nc.vector.tensor_tensor(out=v3, in0=v3, in1=sbc, op=mybir.AluOpType.mult)
                            nc.vector.tensor_scalar(out=vq32[:tn], in0=vq32[:tn], scalar1=EXP2_23, scalar2=EXP2_23,
                                                    op0=mybir.AluOpType.add, op1=mybir.AluOpType.subtract)
                            nc.vector.tensor_tensor(out=virs[i][t][:tn, ch * 384:(ch + 1) * 384].rearrange("p (h d) -> p h d", h=6),
                                                    in0=v3, in1=rbc, op=mybir.AluOpType.mult)
                    # ---- c stats -> cbc[p, h] = c(b, h) broadcast over partitions ----
                    cst = psP.tile([2, 12], F32, tag="P", name="cst")
                    nc.tensor.matmul(cst[:], sels[:], absc[i][:], start=True, stop=True)
                    css = bpool.tile([2, 12], F32, tag="css")
                    nc.vector.tensor_copy(css[:], cst[:])
                    csb = bpool.tile([2, 6], F32, tag="csb")
                    nc.vector.tensor_tensor(out=csb[:], in0=css[:2, 0:6], in1=css[:2, 6:12], op=mybir.AluOpType.mult)
                    nc.vector.tensor_scalar_mul(csb[:], csb[:], C0)
                    nc.sync.dma_start(out=cscr.ap()[b].rearrange("(r j) -> r j", r=2), in_=csb[:])
                    cbc[i] = bpool.tile([128, 12], F32, tag=f"cbc{i}", name=f"cbc{i}")
                    nc.gpsimd.dma_start(out=cbc[i][:], in_=AP(tensor=cscr, offset=b * 12, ap=[[0, 128], [1, 12]]))
                    attnT[i] = bpool.tile([128, 6, NT], BF16, tag=f"attnT{i}", name=f"attnT{i}")

                # ---- attention per (head-pair, batch item), software-pipelined ----
                # s1(u): S^T matmuls -> exp(c*S^T) f32 -> *exp(bias) -> f16 eF
                # s2a(u-L1): Z = onesT(1/255) @ eF -> rzb = recipfast
                # s2b(u-L1-L2): pqT = RNE(eF*rzb) -> PV -> evac
                units = [(jp, i) for jp in range(6) for i in range(2)]
                L1, L2 = 2, 2
                stash = {}
                for u in range(len(units) + L1 + L2):
                    if u < len(units):
                        jp, i = units[u]
                        eps = [hpool.tile([128, c2], F32, tag=f"eps{mt}", name=f"eps{mt}", bufs=2) for mt in range(2)]
                        eF = [hpool.tile([128, c2], F16, tag=f"eF{mt}", name=f"eF{mt}", bufs=L1 + L2 + 1) for mt in range(2)]
                        for mt in range(2):
                            mc = ntl[mt]
                            for hh in range(2):
                                h = 2 * jp + hh
                                base = 64 * hh
                                ps1 = psS.tile([128, NT], F32, tag=f"S{hh}{mt}", name=f"ps{hh}{mt}")
                                # cidx mapping: head h -> cbc column (h%2)*6 + h//2
                                cidx = (h % 2) * 6 + h // 2
                                nc.tensor.matmul(ps1[:mc],
                                                 sgn[i][base:base + 64, 6 + jp, noff[mt]:noff[mt] + mc],
                                                 sgn[i][base:base + 64, jp, :],
                                                 start=True, stop=True)
                                # eps = exp(c * S^T) straight from PSUM
                                nc.scalar.activation(out=eps[mt][:mc, hh * NT:(hh + 1) * NT], in_=ps1[:mc],
                                                     func=mybir.ActivationFunctionType.Exp,
                                                     scale=cbc[i][:mc, cidx:cidx + 1])
                            # eF = f16(eps * exp(bias)) (paired, 394-wide SBUF)
                            nc.vector.tensor_tensor(out=eF[mt][:mc], in0=eps[mt][:mc],
                                                    in1=ebp[mt][:mc, jp, :], op=mybir.AluOpType.mult)
                        stash[u] = [eF, None]
                    if L1 <= u < len(units) + L1:
                        eF = stash[u - L1][0]
                        # Zb = ones(1/255)^T @ eF -> Z/255 replicated on all partitions
                        pz = psZ.tile([128, c2], F32, tag="Z", name="pz")
                        for mt in range(2):
                            mc = ntl[mt]
                            nc.tensor.matmul(pz[:], ones[:mc, :], eF[mt][:mc],
                                             start=(mt == 0), stop=(mt == 1))
                        rzb = hpool.tile([128, c2], F32, tag="rzb", bufs=L2 + 1)
                        nc.vector.reciprocal_approx_fast(out=rzb[:], in_=pz[:])
                        stash[u - L1][1] = rzb
                    if u >= L1 + L2:
                        jp, i = units[u - L1 - L2]
                        eF, rzb = stash.pop(u - L1 - L2)
                        # pqT = RNE(eF * rzb) -> fp16 (exact ints)
                        pqT = [hpool.tile([128, c2], F16, tag=f"pqT{mt}", name=f"pqT{mt}") for mt in range(2)]
                        for mt in range(2):
                            mc = ntl[mt]
                            y2 = hpool.tile([128, c2], F32, tag=f"y2{mt}", name=f"y2{mt}")
                            nc.vector.tensor_tensor(out=y2[:mc], in0=eF[mt][:mc], in1=rzb[:mc],
                                                    op=mybir.AluOpType.mult)
                            nc.vector.tensor_scalar(out=pqT[mt][:mc], in0=y2[:mc], scalar1=EXP2_23, scalar2=EXP2_23,
                                                    op0=mybir.AluOpType.add, op1=mybir.AluOpType.subtract)
                        # PV: both heads into one [128, NT] bank (partition-split)
                        ppv = psP.tile([128, NT], F32, tag="P", name="ppv")
                        for hh in range(2):
                            h = 2 * jp + hh
                            for mt in range(2):
                                mc = ntl[mt]
                                nc.tensor.matmul(ppv[64 * hh:64 * hh + 64, :],
                                                 virs[i][mt][:mc, h * 64:(h + 1) * 64],
                                                 pqT[mt][:mc, hh * NT:(hh + 1) * NT],
                                                 start=(mt == 0), stop=(mt == 1))
                        nc.scalar.activation(out=attnT[i][:, jp, :], in_=ppv[:],
                                             func=mybir.ActivationFunctionType.Copy)
                # ---- proj ----
                for i in range(2):
                    b = bb * 2 + i
                    tb = b * NT
                    osb = [bpool.tile([128, DIM], F32, tag=f"osb{t}", name=f"osb{t}") for t in range(2)]
                    for t in range(2):
                        tn = ntl[t]
                        for ch in range(2):
                            pp = psS.tile([128, 384], F32, tag=f"S{ch % 2}1", name="pp")
                            for jt in range(6):
                                nc.tensor.matmul(pp[:tn], attnT[i][:, jt, noff[t]:noff[t] + tn],
                                                 pws[:, jt, ch * 384:(ch + 1) * 384], start=(jt == 0), stop=(jt == 5))
                            nc.vector.scalar_tensor_tensor(out=osb[t][:tn, ch * 384:(ch + 1) * 384], in0=pp[:tn],
                                                           scalar=1.0, in1=pbs[:tn, ch * 384:(ch + 1) * 384],
                                                           op0=mybir.AluOpType.mult, op1=mybir.AluOpType.add)
                        nc.sync.dma_start(out=d["out"][tb + noff[t]:tb + noff[t] + tn, :], in_=osb[t][:tn])
    nc.compile()
    return nc


def kernel(x, qkv_w, proj_w, proj_b, rel_bias_table, rel_index):
    x = np.asarray(x, dtype=np.float32)
    qkv_w = np.asarray(qkv_w, dtype=np.float32)
    proj_w = np.asarray(proj_w, dtype=np.float32)
    proj_b = np.asarray(proj_b, dtype=np.float32)
    rel_bias_table = np.asarray(rel_bias_table, dtype=np.float32)
    rel_index = np.asarray(rel_index)

    if "nc" not in _CACHE:
        _CACHE["nc"] = _build_nc()
    nc = _CACHE["nc"]

    W2 = np.ascontiguousarray(qkv_w.T)                      # (768, 2304)
    wh = W2.astype(bf)
    wl = (W2 - wh.astype(np.float32)).astype(bf)
    pw = np.ascontiguousarray(proj_w.T / 255.0).astype(bf)  # fold 1/255
    biasg = rel_bias_table[rel_index].astype(np.float32)    # (197,197,12) [n,m,h]
    # ebp[m, jp, hh*197+n] = exp(bias[h=2jp+hh, n, m])
    ebp = np.ascontiguousarray(
        np.exp(biasg).transpose(1, 2, 0).reshape(NT, 6, 2 * NT))
    sel = np.zeros((128, 2), np.float32)
    sel[:64, 0] = 1.0
    sel[64:, 1] = 1.0
    onesc = np.full((128, 128), 1.0 / 255.0, dtype=np.float16)

    in_maps = []
    for c in range(N_CORES):
        xc = x[c * BP:(c + 1) * BP].reshape(TOK, DIM)
        xT = np.ascontiguousarray(xc.T)                     # (768, 3152)
        xh = xT.astype(bf)
        xl = (xT - xh.astype(np.float32)).astype(bf)
        in_maps.append({
            "xh": xh, "xl": xl, "wh": wh, "wl": wl, "pw": pw,
            "pb": proj_b.astype(np.float32), "ebp": ebp,
            "sel": sel, "ones": onesc,
        })

    global _LAST_IN_MAPS
    _LAST_IN_MAPS = in_maps
    res = run_bass_kernel_spmd(nc, in_maps, list(range(N_CORES)))
    out = np.concatenate(
        [res.results[c]["out"].reshape(BP, NT, DIM) for c in range(N_CORES)], axis=0)
    return out.astype(np.float32)
